# revision 1
# baseline (speedup 1.0000x reference)
"""Self-contained Trainium2 kernel for nn_Attention_24799141167815.

Cosine-similarity attention (Swin-v2 style) with continuous position bias.
Strategy: data-parallel over batch B=8 across the 8 NeuronCores (core b
processes batch element b). Everything except host-side input layout prep
(transposes, CPB-MLP + gather table, scalar folds) runs on device.

Device math per core (batch b), all matmul operands bf16:
  qkT  = wqkT.T @ xT + qkb          (1024 feat, 1024 tok)   [feat-major]
  v    = xT.T @ wvT + vb            (1024 tok, 512)         [tok-major]
  rq   = scale_h/max(||q_h,t||,eps) ; rk = 1/max(||k_h,t||,eps)
  qsT  = qkT[:512] * bcast(rq)      (fold l2norm+temp scale into operand)
  knT  = qkT[512:] * bcast(rk)
  qekn = knT.T @ blockdiag(qe*scale)        (keys, H) per-key score offset
  sT   = knT_h.T @ qsT_h                    (keys, q) scores, transposed
  eT   = exp(sT + qekn_h) * expbias_h       (bias multiplicative post-exp,
                                             exp(bias) precomputed on host;
                                             no max-subtraction: |s|<~20)
  o'   = eT.T @ [v_h | 1]                   (q, 65); col 64 = softmax denom
  outh = o'[:, :64] * recip(o'[:, 64:65])   (per-partition scalar)
  outhT= PE-transpose(outh)                 (feat-major for proj)
  out  = outhT.T @ projwT + projb           (tok, 512)
"""

import os
import numpy as np
import ml_dtypes

import concourse.bass as bass
import concourse.mybir as mybir
import concourse.tile as tile
from concourse import bacc
from concourse.bass_utils import run_bass_kernel_spmd

F32 = mybir.dt.float32
BF16 = mybir.dt.bfloat16
FP16 = mybir.dt.float16
AF = mybir.ActivationFunctionType

B, N, C = 8, 1024, 512
H, HD = 8, 64
NT = N // 128    # 8 token/key tiles
CB = C // 128    # 4 cin blocks
QB = 2           # q blocks of 512
NB_BF16 = np.dtype(ml_dtypes.bfloat16)
NB_FP16 = np.dtype(np.float16)

_CACHE = {}


def _build(stage=4, reps=1):
    nc = bacc.Bacc("TRN2", target_bir_lowering=False)

    xT_d = nc.declare_dram_parameter("xT", [C, N], BF16, isOutput=False)
    wqkT_d = nc.declare_dram_parameter("wqkT", [C, 2 * C], BF16, isOutput=False)
    wvT_d = nc.declare_dram_parameter("wvT", [C, C], BF16, isOutput=False)
    qkb_d = nc.declare_dram_parameter("qkb", [2 * C, 1], F32, isOutput=False)
    vbrow_d = nc.declare_dram_parameter("vbrow", [1, C], F32, isOutput=False)
    qesbd_d = nc.declare_dram_parameter("qesbd", [C, H], BF16, isOutput=False)
    scalepp_d = nc.declare_dram_parameter("scalepp", [2, H // 2], F32, isOutput=False)
    projwT_d = nc.declare_dram_parameter("projwT", [C, C], BF16, isOutput=False)
    projbrow_d = nc.declare_dram_parameter("projbrow", [1, C], F32, isOutput=False)
    expbT_d = nc.declare_dram_parameter("expbT", [H, N, N], BF16, isOutput=False)
    selb_d = nc.declare_dram_parameter("selb", [2, 128], FP16, isOutput=False)
    bsum_d = nc.declare_dram_parameter("bsum", [128, 2], BF16, isOutput=False)
    ident_d = nc.declare_dram_parameter("ident", [128, 128], BF16, isOutput=False)
    ones64_d = nc.declare_dram_parameter("ones64", [1, 64], BF16, isOutput=False)
    out_d = nc.declare_dram_parameter("out", [N, C], F32, isOutput=True)

    with tile.TileContext(nc) as tc:
        with (
            tc.tile_pool(name="persist", bufs=1) as persist,
            tc.tile_pool(name="stream", bufs=3) as stream,
            tc.tile_pool(name="expt", bufs=2) as expt_pool,
            tc.tile_pool(name="small", bufs=4) as small,
            tc.tile_pool(name="ps_big", bufs=3, space="PSUM") as ps_big,
            tc.tile_pool(name="ps_av", bufs=2, space="PSUM") as ps_av,
            tc.tile_pool(name="ps_t", bufs=1, space="PSUM") as ps_t,
            tc.tile_pool(name="ps_small", bufs=2, space="PSUM") as ps_small,
        ):
            # ---------------- load constants / weights ----------------
            xT = persist.tile([128, CB, N], BF16, tag="xT")
            nc.sync.dma_start(out=xT, in_=xT_d.rearrange("(cb p) n -> p cb n", p=128))
            wqkT = persist.tile([128, CB, 2 * C], BF16, tag="wqkT")
            nc.sync.dma_start(
                out=wqkT, in_=wqkT_d.rearrange("(cb p) f -> p cb f", p=128))
            wvT = persist.tile([128, CB, C], BF16, tag="wvT")
            nc.sync.dma_start(out=wvT, in_=wvT_d.rearrange("(cb p) f -> p cb f", p=128))
            projwT = persist.tile([128, CB, C], BF16, tag="projwT")
            nc.sync.dma_start(
                out=projwT, in_=projwT_d.rearrange("(cb p) f -> p cb f", p=128))
            qesbd = persist.tile([128, CB, H], BF16, tag="qesbd")
            nc.sync.dma_start(
                out=qesbd, in_=qesbd_d.rearrange("(cb p) h -> p cb h", p=128))
            qkb = persist.tile([128, 2 * CB], F32, tag="qkb")
            nc.sync.dma_start(
                out=qkb, in_=qkb_d.rearrange("(fb p) one -> p (fb one)", p=128))
            scalepp = persist.tile([2, H // 2], F32, tag="scalepp")
            nc.sync.dma_start(out=scalepp, in_=scalepp_d[:])
            vb_bc = persist.tile([128, C], F32, tag="vb_bc")
            nc.sync.dma_start(out=vb_bc, in_=vbrow_d[:].to_broadcast((128, C)))
            projb_bc = persist.tile([128, C], F32, tag="projb_bc")
            nc.sync.dma_start(out=projb_bc, in_=projbrow_d[:].to_broadcast((128, C)))
            selb = persist.tile([2, 128], FP16, tag="selb")
            nc.sync.dma_start(out=selb, in_=selb_d[:])
            bsum = persist.tile([128, 2], BF16, tag="bsum")
            nc.sync.dma_start(out=bsum, in_=bsum_d[:])
            ident = persist.tile([128, 128], BF16, tag="ident")
            nc.sync.dma_start(out=ident, in_=ident_d[:])
            ones64 = persist.tile([1, 64], BF16, tag="ones64")
            nc.sync.dma_start(out=ones64, in_=ones64_d[:])

            for rep in range(reps):
                # ---------------- A: projections ----------------
                # qkT[feat, tok] for q/k features (feat-major), 8 feat blocks
                qkT = persist.tile([128, 2 * CB, N], BF16, tag="qkT")
                for fb in range(2 * CB):
                    for qb in range(QB):
                        ps = ps_big.tile([128, 512], F32, tag="ps_big")
                        for cb in range(CB):
                            nc.tensor.matmul(
                                ps,
                                wqkT[:, cb, fb * 128:(fb + 1) * 128],
                                xT[:, cb, qb * 512:(qb + 1) * 512],
                                start=(cb == 0), stop=(cb == CB - 1),
                            )
                        # psum -> sbuf with per-partition bias add (ACT)
                        nc.scalar.activation(
                            out=qkT[:, fb, qb * 512:(qb + 1) * 512], in_=ps,
                            func=AF.Identity, bias=qkb[:, fb:fb + 1], scale=1.0)

                # v[tok, feat] tok-major, with ones column per head: [p, tb, h, 65]
                v_sb = persist.tile([128, NT, H, HD + 1], BF16, tag="v_sb")
                nc.vector.memset(v_sb[:, :, :, HD:HD + 1], 1.0)
                for tb in range(NT):
                    ps = ps_big.tile([128, 512], F32, tag="ps_big")
                    for cb in range(CB):
                        nc.tensor.matmul(
                            ps,
                            xT[:, cb, tb * 128:(tb + 1) * 128],
                            wvT[:, cb, :],
                            start=(cb == 0), stop=(cb == CB - 1),
                        )
                    nc.vector.tensor_add(
                        v_sb[:, tb, :, 0:HD],
                        ps.rearrange("p (h d) -> p h d", h=H),
                        vb_bc.rearrange("p (h d) -> p h d", h=H),
                    )

                qsT = knT = outhT = None
                if stage >= 2:
                    # ------------- B/C: fold l2 norms into operands -------------
                    rqs = persist.tile([2, CB, N], FP16, tag="rqs")
                    rks = persist.tile([2, CB, N], FP16, tag="rks")
                    for half, rdst in ((0, rqs), (1, rks)):
                        for fb in range(CB):
                            sq = small.tile([128, N], BF16, tag="sq")
                            src = qkT[:, half * CB + fb, :]
                            nc.vector.tensor_mul(sq, src, src)
                            for qb in range(QB):
                                pss = ps_small.tile([2, 512], F32, tag="ps_small")
                                nc.tensor.matmul(
                                    pss, bsum, sq[:, qb * 512:(qb + 1) * 512],
                                    start=True, stop=True)
                                nrm = small.tile([2, 512], F32, tag="nrm")
                                nc.scalar.activation(
                                    out=nrm, in_=pss, func=AF.Sqrt, bias=0.0, scale=1.0)
                                nc.vector.reciprocal(nrm, nrm)
                                # 1/max(||t||,1e-12) == min(1/||t||, 1e12)
                                nc.vector.tensor_scalar_min(nrm, nrm, 1e12)
                                if half == 0:
                                    nc.vector.tensor_scalar_mul(
                                        rdst[:, fb, qb * 512:(qb + 1) * 512],
                                        nrm, scalepp[:, fb:fb + 1])
                                else:
                                    nc.vector.tensor_copy(
                                        rdst[:, fb, qb * 512:(qb + 1) * 512], nrm)

                    qsT = persist.tile([128, CB, N], BF16, tag="qsT")
                    knT = persist.tile([128, CB, N], BF16, tag="knT")
                    for half, rsrc, dst in ((0, rqs, qsT), (1, rks, knT)):
                        for fb in range(CB):
                            for qb in range(QB):
                                psb = ps_small.tile([128, 512], F32, tag="ps_small")
                                nc.tensor.matmul(
                                    psb, selb,
                                    rsrc[:, fb, qb * 512:(qb + 1) * 512],
                                    start=True, stop=True)
                                nc.vector.tensor_mul(
                                    dst[:, fb, qb * 512:(qb + 1) * 512],
                                    qkT[:, half * CB + fb, qb * 512:(qb + 1) * 512],
                                    psb)

                    # qekn[key, h] = sum_f knT[f, key] * qesbd[f, h]; per key-tile
                    qekn = persist.tile([128, NT, H], F32, tag="qekn")
                    for kt in range(NT):
                        psq = ps_small.tile([128, H], F32, tag="ps_small")
                        for fb in range(CB):
                            nc.tensor.matmul(
                                psq,
                                knT[:, fb, kt * 128:(kt + 1) * 128],
                                qesbd[:, fb, :],
                                start=(fb == 0), stop=(fb == CB - 1),
                            )
                        nc.vector.tensor_copy(qekn[:, kt, :], psq)

                if stage >= 3:
                    # ---------------- D: attention ----------------
                    outhT = persist.tile([128, CB, N], BF16, tag="outhT")
                    for hp in range(H // 2):
                        for qb in range(QB):
                            for sub in range(2):
                                h = 2 * hp + sub
                                po = sub * 64  # partition offset in feat block hp
                                expb = stream.tile([128, NT, 512], BF16, tag="expb")
                                for kt in range(NT):
                                    nc.sync.dma_start(
                                        out=expb[:, kt, :],
                                        in_=expbT_d[h, kt * 128:(kt + 1) * 128,
                                                    qb * 512:(qb + 1) * 512])
                                eT = expt_pool.tile([128, NT, 512], BF16, tag="eT")
                                for kt in range(NT):
                                    pss = ps_big.tile([128, 512], F32, tag="ps_big")
                                    nc.tensor.matmul(
                                        pss,
                                        knT[:, hp, kt * 128:(kt + 1) * 128][po:po + 64],
                                        qsT[:, hp, qb * 512:(qb + 1) * 512][po:po + 64],
                                        start=True, stop=True,
                                    )
                                    etmp = small.tile([128, 512], BF16, tag="etmp")
                                    nc.scalar.activation(
                                        out=etmp, in_=pss, func=AF.Exp,
                                        bias=qekn[:, kt, h:h + 1], scale=1.0)
                                    mul_eng = nc.vector if kt % 2 == 0 else nc.gpsimd
                                    mul_eng.tensor_mul(
                                        eT[:, kt, :], etmp, expb[:, kt, :])
                                # AV in (hd, q): out'T (65, 512), feat-major
                                pavT = ps_av.tile([HD + 1, 512], F32, tag="ps_av")
                                for kt in range(NT):
                                    nc.tensor.matmul(
                                        pavT,
                                        v_sb[:, kt, h, :],
                                        eT[:, kt, :],
                                        start=(kt == 0), stop=(kt == NT - 1),
                                    )
                                avs = small.tile([HD, 512], BF16, tag="avs")
                                nc.vector.tensor_copy(avs, pavT[0:HD, :])
                                rrec = small.tile([1, 512], BF16, tag="rrec")
                                with nc.allow_low_precision(
                                        reason="softmax denom recip in bf16"):
                                    nc.vector.reciprocal(
                                        rrec, pavT[HD:HD + 1, :])
                                rrb = ps_small.tile([HD, 512], F32, tag="ps_small")
                                nc.tensor.matmul(rrb, ones64, rrec,
                                                 start=True, stop=True)
                                nc.vector.tensor_mul(
                                    outhT[:, hp, qb * 512:(qb + 1) * 512][po:po + 64],
                                    avs, rrb)

                if stage >= 4:
                    # ---------------- E: output projection ----------------
                    for tb in range(NT):
                        ps = ps_big.tile([128, 512], F32, tag="ps_big")
                        for fb in range(CB):
                            nc.tensor.matmul(
                                ps,
                                outhT[:, fb, tb * 128:(tb + 1) * 128],
                                projwT[:, fb, :],
                                start=(fb == 0), stop=(fb == CB - 1),
                            )
                        osb = stream.tile([128, C], F32, tag="osb")
                        nc.vector.tensor_add(osb, ps, projb_bc)
                        nc.sync.dma_start(
                            out=out_d[tb * 128:(tb + 1) * 128, :], in_=osb)
                else:
                    dbg = outhT if stage >= 3 else (knT if stage >= 2 else qkT)
                    for tb in range(NT):
                        osb = stream.tile([128, C], F32, tag="osb")
                        nc.vector.tensor_copy(osb, dbg[:, tb % CB, 0:C])
                        nc.sync.dma_start(
                            out=out_d[tb * 128:(tb + 1) * 128, :], in_=osb)

    nc.compile()
    return nc


def _host_prep(inputs):
    """Host-side layout/scalar prep. Returns per-core input maps."""
    x = np.asarray(inputs["x"], dtype=np.float32)
    qkv_w = np.asarray(inputs["qkv_w"], dtype=np.float32)
    qkv_b = np.asarray(inputs["qkv_b"], dtype=np.float32)
    proj_w = np.asarray(inputs["proj_w"], dtype=np.float32)
    proj_b = np.asarray(inputs["proj_b"], dtype=np.float32)
    temp = np.asarray(inputs["temperature"], dtype=np.float32).reshape(H)
    qe = np.asarray(inputs["query_embedding"], dtype=np.float32).reshape(H, HD)
    tab = np.asarray(inputs["relative_coords_table"], dtype=np.float32)
    idx = np.asarray(inputs["relative_pos_index"])
    f1w = np.asarray(inputs["cpb_fc1_w"], dtype=np.float32)
    f1b = np.asarray(inputs["cpb_fc1_b"], dtype=np.float32)
    f2w = np.asarray(inputs["cpb_fc2_w"], dtype=np.float32)
    f2b = np.asarray(inputs["cpb_fc2_b"], dtype=np.float32)
    sls = np.asarray(inputs["seq_length_scale"], dtype=np.float32)

    # softplus(temperature) * seq_length_scale
    scale = (np.logaddexp(0.0, temp) * sls[0]).astype(np.float32)

    # continuous position bias table -> gathered, transposed, exponentiated
    hidden = np.maximum(tab @ f1w.T + f1b, 0.0)
    bias_tab = (hidden @ f2w.T + f2b).astype(np.float32)      # (T, H)
    bias = bias_tab[idx]                                       # (q, k, H)
    expbT = np.exp(np.transpose(bias, (2, 1, 0)))              # (H, k, q)
    expbT = np.ascontiguousarray(expbT).astype(NB_BF16)

    wqkT = np.ascontiguousarray(qkv_w[:2 * C].T).astype(NB_BF16)   # (cin, 1024)
    wvT = np.ascontiguousarray(qkv_w[2 * C:].T).astype(NB_BF16)    # (cin, 512)
    projwT = np.ascontiguousarray(proj_w.T).astype(NB_BF16)        # (cin, 512)
    qkb = qkv_b[:2 * C].reshape(2 * C, 1).copy()
    vbrow = qkv_b[2 * C:].reshape(1, C).copy()
    projbrow = proj_b.reshape(1, C).copy()
    qesbd = np.zeros((C, H), dtype=np.float32)
    for h in range(H):
        qesbd[h * HD:(h + 1) * HD, h] = qe[h] * scale[h]
    qesbd = qesbd.astype(NB_BF16)
    scalepp = np.ascontiguousarray(scale.reshape(H // 2, 2).T)

    selb = np.zeros((2, 128), dtype=NB_FP16)
    selb[0, :64] = 1.0
    selb[1, 64:] = 1.0
    bsum = np.zeros((128, 2), dtype=NB_BF16)
    bsum[:64, 0] = 1.0
    bsum[64:, 1] = 1.0
    ident = np.eye(128, dtype=NB_BF16)
    ones64 = np.ones((1, 64), dtype=NB_BF16)

    shared = dict(
        wqkT=wqkT, wvT=wvT, qkb=qkb, vbrow=vbrow, qesbd=qesbd,
        scalepp=scalepp, projwT=projwT, projbrow=projbrow, expbT=expbT,
        selb=selb, bsum=bsum, ident=ident, ones64=ones64,
    )
    in_maps = []
    for b in range(B):
        m = dict(shared)
        m["xT"] = np.ascontiguousarray(x[b].T).astype(NB_BF16)
        in_maps.append(m)
    return in_maps


def get_nc(reps=1):
    key = ("nc", reps)
    if key not in _CACHE:
        stage = int(os.environ.get("BASS_STAGE", "4"))
        _CACHE[key] = _build(stage, reps)
    return _CACHE[key]


def kernel(**inputs) -> np.ndarray:
    nc = get_nc()
    in_maps = _host_prep(inputs)
    res = run_bass_kernel_spmd(nc, in_maps, core_ids=list(range(B)))
    out = np.stack([res.results[b]["out"] for b in range(B)], axis=0)
    return out.astype(np.float32)



# revision 12
# speedup vs baseline: 1.3307x; 1.3307x over previous
"""Self-contained Trainium2 kernel for nn_Attention_24799141167815.

Cosine-similarity attention (Swin-v2 style) with continuous position bias.
Data-parallel over batch B=8 across 8 NeuronCores (core b handles batch b).

Device math per core (batch b):
  A:  qkT = wqkT.T @ xT          (1024 feat x 1024 tok, feat-major, raw)
      v   = xT.T @ wvT           (tok-major [tok, h, 64] + ones col; v-bias
                                  folded into proj bias on host)
  B:  ss[16, tok] = sum of squares per (q/k head, tok) via scatter-matmul
      rnorm = sqrt(scale_h^2 * min(1/ss, 1e24))   (q rows scaled, k rows raw)
      rnb   = DMA-broadcast of q rows -> [128, fb, tok]
      qsT   = qkT_q * rnb        (l2-normalized, temperature-scaled q)
      rkT   = PE-transpose of k rows of rnorm -> [tok-part, kt, 8] (ACT scale)
      qekn  = (qkT_k.T @ qesbd) * rkT    per-key bias  (query_embedding term)
  D:  per (h, kt): s_raw = qkT_k_h.T @ qsT_h          ([128 key, 1024 q])
      etmp = exp(s_raw * rkT + qekn)                  (ACT, scale+bias fused)
      eT   = etmp * expb_h                            (DVE / GpSimd split)
      pav += v_h.T @ eT                               ([65, 1024], row 64 = denom)
      rrec = recip_approx_fast(pav[64]); rrb = DMA-broadcast -> [64, 1024]
      outhT_h = pav[0:64] * rrb                       (fused psum-read + divide)
  E:  outT = projwT.T @ outhT + projb'    (c-major [512, 1024]; host transposes)
"""

import os
import numpy as np
import ml_dtypes

import concourse.bass as bass
import concourse.mybir as mybir
import concourse.tile as tile
from concourse import bacc
from concourse.bass_utils import run_bass_kernel_spmd

F32 = mybir.dt.float32
BF16 = mybir.dt.bfloat16
AF = mybir.ActivationFunctionType

B, N, C = 8, 1024, 512
H, HD = 8, 64
NT = N // 128     # 8 token/key tiles
CB = C // 128     # 4 cin blocks
FB = 2 * CB       # 8 q+k feature blocks
NB_BF16 = np.dtype(ml_dtypes.bfloat16)

_CACHE = {}


def _build(stage=4):
    nc = bacc.Bacc("TRN2", target_bir_lowering=False)

    xT_d = nc.declare_dram_parameter("xT", [C, N], BF16, isOutput=False)
    wqkT_d = nc.declare_dram_parameter("wqkT", [C, 2 * C], BF16, isOutput=False)
    wvT_d = nc.declare_dram_parameter("wvT", [C, C], BF16, isOutput=False)
    qkb_d = nc.declare_dram_parameter("qkb", [2 * C, 1], F32, isOutput=False)
    qesbd_d = nc.declare_dram_parameter("qesbd", [C, H], BF16, isOutput=False)
    scalesq_d = nc.declare_dram_parameter("scalesq", [40, 1], F32, isOutput=False)
    projwT_d = nc.declare_dram_parameter("projwT", [C, C], BF16, isOutput=False)
    projb_d = nc.declare_dram_parameter("projb", [C, 1], F32, isOutput=False)
    ssum16_d = nc.declare_dram_parameter("ssum16", [2 * C, 40], BF16, isOutput=False)
    ident_d = nc.declare_dram_parameter("ident", [128, 128], BF16, isOutput=False)
    expbT_d = nc.declare_dram_parameter("expbT", [H, N, N], BF16, isOutput=False)
    outT_d = nc.declare_dram_parameter("outT", [C, N], F32, isOutput=True)

    with tile.TileContext(nc) as tc:
        with (
            tc.tile_pool(name="persist", bufs=1) as persist,
            tc.tile_pool(name="expbp", bufs=2) as expbp,
            tc.tile_pool(name="etp", bufs=3) as etp,
            tc.tile_pool(name="eTp", bufs=3) as eTp,
            tc.tile_pool(name="sqp", bufs=2) as sqp,
            tc.tile_pool(name="rrp", bufs=2) as rrp,
            tc.tile_pool(name="osp", bufs=2) as osp,
            tc.tile_pool(name="dram", bufs=2, space="DRAM") as dram,
            tc.tile_pool(name="ps_qk", bufs=2, space="PSUM") as ps_qk,
            tc.tile_pool(name="ps_av", bufs=2, space="PSUM") as ps_av,
        ):
            # ---------------- load constants / weights (ACT hwdge ring) ------
            xT = persist.tile([128, CB, N], BF16, tag="xT")
            nc.scalar.dma_start(out=xT, in_=xT_d.rearrange("(cb p) n -> p cb n", p=128))
            wqkT = persist.tile([128, CB, 2 * C], BF16, tag="wqkT")
            nc.scalar.dma_start(
                out=wqkT, in_=wqkT_d.rearrange("(cb p) f -> p cb f", p=128))
            wvT = persist.tile([128, CB, C], BF16, tag="wvT")
            nc.scalar.dma_start(out=wvT, in_=wvT_d.rearrange("(cb p) f -> p cb f", p=128))
            projwT = persist.tile([128, CB, C], BF16, tag="projwT")
            nc.scalar.dma_start(
                out=projwT, in_=projwT_d.rearrange("(cb p) f -> p cb f", p=128))
            qesbd = persist.tile([128, CB, H], BF16, tag="qesbd")
            nc.scalar.dma_start(
                out=qesbd, in_=qesbd_d.rearrange("(cb p) h -> p cb h", p=128))
            qkb = persist.tile([128, FB], F32, tag="qkb")
            nc.scalar.dma_start(
                out=qkb, in_=qkb_d.rearrange("(fb p) one -> p (fb one)", p=128))
            projb = persist.tile([128, CB], F32, tag="projb")
            nc.scalar.dma_start(
                out=projb, in_=projb_d.rearrange("(cb p) one -> p (cb one)", p=128))
            scalesq = persist.tile([40, 1], F32, tag="scalesq")
            nc.scalar.dma_start(out=scalesq, in_=scalesq_d[:])
            ssum16 = persist.tile([128, FB, 40], BF16, tag="ssum16")
            nc.scalar.dma_start(
                out=ssum16, in_=ssum16_d.rearrange("(fb p) r -> p fb r", p=128))
            ident = persist.tile([128, 128], BF16, tag="ident")
            nc.scalar.dma_start(out=ident, in_=ident_d[:])

            # ---------------- A: projections ----------------
            # qkT[feat, tok] raw q+k features, with q/k bias added on copy
            qkT = persist.tile([128, FB, N], BF16, tag="qkT")
            for fb in range(FB):
                ps = ps_qk.tile([128, N], F32, tag="qk")
                for qb in range(2):
                    for cb in range(CB):
                        nc.tensor.matmul(
                            ps[:, qb * 512:(qb + 1) * 512],
                            wqkT[:, cb, fb * 128:(fb + 1) * 128],
                            xT[:, cb, qb * 512:(qb + 1) * 512],
                            start=(cb == 0), stop=(cb == CB - 1),
                        )
                # psum -> sbuf with per-partition bias add; split ACT/DVE
                if fb % 2 == 0:
                    nc.scalar.activation(
                        out=qkT[:, fb, :], in_=ps, func=AF.Identity,
                        bias=qkb[:, fb:fb + 1], scale=1.0)
                else:
                    nc.vector.tensor_scalar_add(qkT[:, fb, :], ps, qkb[:, fb:fb + 1])

            # v[tok, h, 65] tok-major; col 64 = ones (softmax denominator)
            v_sb = persist.tile([128, NT, H, HD + 1], BF16, tag="v_sb")
            nc.vector.memset(v_sb[:, :, :, HD:HD + 1], 1.0)
            for tb in range(NT):
                ps = ps_av.tile([128, C], F32, tag="av")
                for cb in range(CB):
                    nc.tensor.matmul(
                        ps,
                        xT[:, cb, tb * 128:(tb + 1) * 128],
                        wvT[:, cb, :],
                        start=(cb == 0), stop=(cb == CB - 1),
                    )
                nc.vector.tensor_copy(
                    v_sb[:, tb, :, 0:HD], ps.rearrange("p (h d) -> p h d", h=H))

            if stage < 2:
                _debug_out(nc, osp, qkT, outT_d)
            # ---------------- B: norms ----------------
            qsT = persist.tile([128, CB, N], BF16, tag="qsT")
            rkT = persist.tile([128, NT, H], F32, tag="rkT")
            qekn = persist.tile([128, NT, H], F32, tag="qekn")
            if stage >= 2:
                # ss[40, tok]: rows 0-7 q-head sumsq, 32-39 k-head sumsq
                ss_ps = ps_qk.tile([40, N], F32, tag="qk")
                for fb in range(FB):
                    sq = sqp.tile([128, N], BF16, tag="sq")
                    nc.vector.tensor_mul(sq, qkT[:, fb, :], qkT[:, fb, :])
                    for qb in range(2):
                        nc.tensor.matmul(
                            ss_ps[:, qb * 512:(qb + 1) * 512],
                            ssum16[:, fb, :],
                            sq[:, qb * 512:(qb + 1) * 512],
                            start=(fb == 0), stop=(fb == FB - 1),
                        )
                rinv = persist.tile([40, N], F32, tag="rinv")
                nc.vector.reciprocal_approx_fast(rinv, ss_ps)
                nc.vector.tensor_scalar_min(rinv, rinv, 1e24)
                # rnorm = sqrt(rinv * scale^2):  scale_h/||q||  resp. 1/||k||
                rnorm = persist.tile([40, N], BF16, tag="rnorm")
                nc.scalar.activation(
                    out=rnorm, in_=rinv, func=AF.Sqrt, bias=0.0,
                    scale=scalesq[:, 0:1])

                # rnb[tok-part(2x64), fb, tok]: q rows broadcast across parts
                # (round-trip through DRAM: SBUF-source broadcast is illegal)
                rnq_d = dram.tile([8, N], BF16, tag="rnq_d")
                nc.sync.dma_start(out=rnq_d, in_=rnorm[0:8, :])
                rnb = persist.tile([128, CB, N], BF16, tag="rnb")
                for f in range(CB):
                    for phi in range(2):
                        nc.sync.dma_start(
                            out=rnb[64 * phi:64 * (phi + 1), f, :],
                            in_=rnq_d[2 * f + phi:2 * f + phi + 1, :]
                            .to_broadcast((64, N)))
                # qsT = qkT_q * rnb  (scaled l2-normalized q)
                for f in range(CB):
                    nc.vector.tensor_mul(qsT[:, f, :], qkT[:, f, :], rnb[:, f, :])

                # rkT[tok-part, kt, h]: transpose of k rows of rnorm (f32)
                for kt in range(NT):
                    pt = ps_av.tile([128, 8], BF16, tag="av")
                    nc.tensor.transpose(
                        pt, rnorm[32:40, kt * 128:(kt + 1) * 128],
                        ident[32:40, 32:40])
                    nc.scalar.activation(
                        out=rkT[:, kt, :], in_=pt, func=AF.Identity,
                        bias=0.0, scale=1.0)
                    # qekn = (qkT_k.T @ qesbd) * rkT  (query-embedding bias)
                    pq = ps_av.tile([128, H], F32, tag="av")
                    for cb in range(CB):
                        nc.tensor.matmul(
                            pq,
                            qkT[:, CB + cb, kt * 128:(kt + 1) * 128],
                            qesbd[:, cb, :],
                            start=(cb == 0), stop=(cb == CB - 1),
                        )
                    nc.vector.tensor_mul(qekn[:, kt, :], pq, rkT[:, kt, :])

            if stage == 2:
                _debug_out(nc, osp, qsT, outT_d)
            # ---------------- D: attention ----------------
            outhT = persist.tile([128, CB, N], BF16, tag="outhT")
            for h in range(H if stage >= 3 else 0):
                hp, po = h // 2, (h % 2) * 64
                expb = expbp.tile([128, NT, N], BF16, tag="expb")
                nc.sync.dma_start(
                    out=expb, in_=expbT_d[h].rearrange("(kt p) q -> p kt q", p=128))
                pav = ps_av.tile([HD + 1, N], F32, tag="av")
                for kt in range(NT):
                    pss = ps_qk.tile([128, N], F32, tag="qk")
                    for qb in range(2):
                        nc.tensor.matmul(
                            pss[:, qb * 512:(qb + 1) * 512],
                            qkT[:, CB + hp, kt * 128:(kt + 1) * 128][po:po + 64],
                            qsT[:, hp, qb * 512:(qb + 1) * 512][po:po + 64],
                            start=True, stop=True,
                        )
                    etmp = etp.tile([128, N], BF16, tag="etmp")
                    nc.scalar.activation(
                        out=etmp, in_=pss, func=AF.Exp,
                        bias=qekn[:, kt, h:h + 1], scale=rkT[:, kt, h:h + 1])
                    eT = eTp.tile([128, N], BF16, tag="eT")
                    mul_eng = nc.gpsimd if (h * NT + kt) % 3 == 2 else nc.vector
                    mul_eng.tensor_mul(eT, etmp, expb[:, kt, :])
                    for qb in range(2):
                        nc.tensor.matmul(
                            pav[:, qb * 512:(qb + 1) * 512],
                            v_sb[:, kt, h, :],
                            eT[:, qb * 512:(qb + 1) * 512],
                            start=(kt == 0), stop=(kt == NT - 1),
                        )
                # softmax division: recip of denom row, broadcast, fused mul
                # (stage denom to a base-0 SBUF tile: custom-DVE ops mishandle
                #  partition-offset inputs)
                dtmp = rrp.tile([1, N], F32, tag="dtmp")
                nc.scalar.copy(dtmp, pav[HD:HD + 1, :])
                rrec = rrp.tile([1, N], F32, tag="rrec")
                nc.vector.reciprocal_approx_fast(rrec, dtmp)
                rrec_d = dram.tile([1, N], F32, tag="rrec_d")
                nc.sync.dma_start(out=rrec_d, in_=rrec)
                rrb = rrp.tile([64, N], F32, tag="rrb")
                nc.sync.dma_start(out=rrb, in_=rrec_d[0:1, :].to_broadcast((64, N)))
                nc.vector.tensor_mul(
                    outhT[:, hp, :][po:po + 64], pav[0:HD, :], rrb)

            if stage == 3:
                _debug_out(nc, osp, outhT, outT_d)
            # ---------------- E: output projection (c-major) ----------------
            for cc in range(CB if stage >= 4 else 0):
                ps = ps_qk.tile([128, N], F32, tag="qk")
                for th in range(2):
                    for fb in range(CB):
                        nc.tensor.matmul(
                            ps[:, th * 512:(th + 1) * 512],
                            projwT[:, fb, cc * 128:(cc + 1) * 128],
                            outhT[:, fb, th * 512:(th + 1) * 512],
                            start=(fb == 0), stop=(fb == CB - 1),
                        )
                osb = osp.tile([128, N], F32, tag="osb")
                nc.vector.tensor_scalar_add(osb, ps, projb[:, cc:cc + 1])
                nc.scalar.dma_start(
                    out=outT_d[cc * 128:(cc + 1) * 128, :], in_=osb)

    nc.compile()
    return nc


def _debug_out(nc, osp, dbg, outT_d):
    for cc in range(CB):
        osb = osp.tile([128, N], F32, tag="osb")
        nc.vector.tensor_copy(osb, dbg[:, cc, :])
        nc.scalar.dma_start(out=outT_d[cc * 128:(cc + 1) * 128, :], in_=osb)


def _host_prep(inputs):
    """Host-side layout/scalar prep. Returns per-core input maps."""
    x = np.asarray(inputs["x"], dtype=np.float32)
    qkv_w = np.asarray(inputs["qkv_w"], dtype=np.float32)
    qkv_b = np.asarray(inputs["qkv_b"], dtype=np.float32)
    proj_w = np.asarray(inputs["proj_w"], dtype=np.float32)
    proj_b = np.asarray(inputs["proj_b"], dtype=np.float32)
    temp = np.asarray(inputs["temperature"], dtype=np.float32).reshape(H)
    qe = np.asarray(inputs["query_embedding"], dtype=np.float32).reshape(H, HD)
    tab = np.asarray(inputs["relative_coords_table"], dtype=np.float32)
    idx = np.asarray(inputs["relative_pos_index"])
    f1w = np.asarray(inputs["cpb_fc1_w"], dtype=np.float32)
    f1b = np.asarray(inputs["cpb_fc1_b"], dtype=np.float32)
    f2w = np.asarray(inputs["cpb_fc2_w"], dtype=np.float32)
    f2b = np.asarray(inputs["cpb_fc2_b"], dtype=np.float32)
    sls = np.asarray(inputs["seq_length_scale"], dtype=np.float32)

    # softplus(temperature) * seq_length_scale
    scale = (np.logaddexp(0.0, temp) * sls[0]).astype(np.float32)

    # continuous position bias table -> gathered, transposed, exponentiated
    hidden = np.maximum(tab @ f1w.T + f1b, 0.0)
    bias_tab = (hidden @ f2w.T + f2b).astype(np.float32)      # (T, H)
    bias = bias_tab[idx]                                       # (q, k, H)
    expbT = np.exp(np.transpose(bias, (2, 1, 0)))              # (H, k, q)
    expbT = np.ascontiguousarray(expbT).astype(NB_BF16)

    wqkT = np.ascontiguousarray(qkv_w[:2 * C].T).astype(NB_BF16)   # (cin, 1024)
    wvT = np.ascontiguousarray(qkv_w[2 * C:].T).astype(NB_BF16)    # (cin, 512)
    projwT = np.ascontiguousarray(proj_w.T).astype(NB_BF16)        # (cin, 512)
    qkb = qkv_b[:2 * C].reshape(2 * C, 1).copy()
    vb = qkv_b[2 * C:]
    # fold v-bias through the projection:  (o + vb) @ W.T + b = o@W.T + b'
    projb = (proj_b + vb @ proj_w.T).reshape(C, 1).astype(np.float32)
    qesbd = np.zeros((C, H), dtype=np.float32)
    for h in range(H):
        qesbd[h * HD:(h + 1) * HD, h] = qe[h] * scale[h]
    qesbd = qesbd.astype(NB_BF16)
    # rows 0-7: q-heads get scale_h^2 inside the sqrt; rows 32-39 (k): 1.0
    scalesq = np.ones(40, np.float32)
    scalesq[0:8] = scale * scale
    scalesq = scalesq.reshape(40, 1).astype(np.float32)

    # scatter-stationaries for per-head sumsq: [fb][128, 40]
    ssum16 = np.zeros((FB, 128, 40), dtype=NB_BF16)
    for f in range(FB):
        base = 0 if f < CB else 32
        j = f if f < CB else f - CB
        ssum16[f, 0:64, base + 2 * j] = 1.0
        ssum16[f, 64:128, base + 2 * j + 1] = 1.0
    ssum16 = np.ascontiguousarray(ssum16).reshape(2 * C, 40)
    ident = np.eye(128, dtype=NB_BF16)

    shared = dict(
        wqkT=wqkT, wvT=wvT, qkb=qkb, qesbd=qesbd, scalesq=scalesq,
        projwT=projwT, projb=projb, ssum16=ssum16, ident=ident, expbT=expbT,
    )
    in_maps = []
    for b in range(B):
        m = dict(shared)
        m["xT"] = np.ascontiguousarray(x[b].T).astype(NB_BF16)
        in_maps.append(m)
    return in_maps


def _assemble(res):
    """Gather per-core c-major outputs into the full (B, N, C) result."""
    return np.stack(
        [np.ascontiguousarray(res.results[b]["outT"].T) for b in range(B)],
        axis=0).astype(np.float32)


def get_nc():
    key = "nc"
    if key not in _CACHE:
        stage = int(os.environ.get("BASS_STAGE", "4"))
        _CACHE[key] = _build(stage)
    return _CACHE[key]


def kernel(**inputs) -> np.ndarray:
    nc = get_nc()
    in_maps = _host_prep(inputs)
    res = run_bass_kernel_spmd(nc, in_maps, core_ids=list(range(B)))
    return _assemble(res)
